# revision 65
# baseline (speedup 1.0000x reference)
"""Trainium2 Bass kernel for a single attention head.

Problem: X[4,4096,1024], Wq/Wk/Wv[1024,128] ->
  softmax((X@Wq)(X@Wk)^T / sqrt(1024)) @ (X@Wv)   -> [4,4096,128]

Sharding: 8 cores = 4 batches x 2 query-halves. Each core receives the full
X of its batch (rolled so its query half is rows [0:2048)), computes K/V for
all 4096 keys and flash-style attention for its 2048 queries.

Pipeline (all matmuls bf16 inputs, fp32 PSUM accumulation):
  - X^T is pre-laid-out and rounded to bf16 on the host (pure relayout),
    so the device does plain chunked DMA loads of X^T. Weights are
    host-prepped to bf16 tiles the same way.
  - Startup: ~24 dummy warmup matmuls (ones x ones into the out PSUM bank,
    overwritten later) run during the initial DMA latency so the PE HAM
    clock-gate is warm (2.4GHz) when real matmuls start; weights are
    DMA'd first and chunks 0-1 arrive in 2-d-tile slices so the first
    projection matmul can start as soon as ~160KB have landed; all
    remaining chunk DMAs are issued up front (the DGE queues drain them
    in order, so data always arrives ahead of the projection consuming
    it).
  - Projections K^T/V^T/Q^T per 512-token chunk with two PSUM banks
    interleaved (K/V pairs); production of chunks 1-7 is interleaved
    into the first attention q-chunk, 4 matmuls per k-step (the qi0
    k-loop is PE-saturated at ~8 matmuls / 1.72us per k-tile).
  - Transposed flash attention, software-pipelined: S^T(kt+1) is issued
    to the PE before O^T(kt) so the PE has work during exp(kt) on ACT.
    The qi1 k-loop is ACT-paced (~1.0us per [128,1024] Exp), so ACT is
    kept free of everything except exp.
  - exp outputs land in a 16-slice ring tile; the softmax denominator
    is accumulated by one contiguous 4-slice [128,4096] DVE add per 4
    k-tiles; the 4 interleaved partials are then tree-reduced by two
    more DVE adds, and only the final cross-partition sum uses a
    ones-matmul on the PE (1 matmul per 512 queries instead of 8).
  - Deferred epilogue: out_ps is evacuated by a DVE copy right after
    the last O matmul; the l -> 1/l -> scale chain for q-chunk 0 runs
    inside q-chunk 1's loop. The final q-chunk skips the evacuation
    (DVE multiplies straight out of PSUM) and runs a quartered
    mul->DMA pipeline; its post-exp(31) dependency chain is one
    O-half + one ones-matmul per half.
  - O^T is DMA'd out transposed and un-transposed on the host.
"""

import numpy as np

B, N, D, H = 4, 4096, 1024, 128
NCORES = 8
QSPLIT = 2  # cores per batch (query halves)
NQ = N // QSPLIT
SCALE = 1.0 / float(np.sqrt(np.float32(D)))
P = 128  # partitions
FB = 512  # matmul free-dim block (one fp32 PSUM bank)
CR = 512  # X rows per projection job
QC = 1024  # query chunk
DT = D // P   # 8 contraction tiles
NT = N // P   # 32 key tiles
NC = N // CR  # 8 projection jobs
XC = 8        # X DMA chunks
XCR = N // XC
KPC = CR // P  # 4 key tiles per chunk
PR = 16       # pT ring depth (slices)
GL = 4        # denominator group length (ring slices per DVE add)
WARM_MM = 12  # HAM warmup dummy matmuls (N=512: self-equalizing span)
UP = DT // 2  # fp8 DoubleRow contraction pairs (256-deep per matmul)
F8K = False   # fp8 K-projections: ~3us PE saving but the extra DMA and
#               trigger scheduling cost more than that on HW, and rel err
#               rises 3.4e-3 -> 1.6e-2; keep the bf16 path.


def emit_attention(tc, XT, Ws, OT, XF8=None, WK8=None, n=N, d=D, nq=NQ):
    """Emit the single-core attention program into TileContext tc."""
    import concourse.mybir as mybir

    nc = tc.nc
    dt = mybir.dt
    f32, bf16, f8 = dt.float32, dt.bfloat16, dt.float8e4
    AF = mybir.ActivationFunctionType
    DR = mybir.MatmulPerfMode.DoubleRow
    qc = QC
    NQC = nq // qc

    from contextlib import ExitStack

    with ExitStack() as ctx:
        cpool = ctx.enter_context(tc.tile_pool(name="const", bufs=1))
        big = ctx.enter_context(tc.tile_pool(name="big", bufs=1))
        vtp = ctx.enter_context(tc.tile_pool(name="vtp", bufs=2))
        gsp = ctx.enter_context(tc.tile_pool(name="gsp", bufs=2))
        epp = ctx.enter_context(tc.tile_pool(name="ep", bufs=2))
        # PSUM: p12 2x1 + stp 2x2 + accp 1x2 = 8 banks
        p12 = ctx.enter_context(tc.tile_pool(name="p12", bufs=2, space="PSUM"))
        stp = ctx.enter_context(tc.tile_pool(name="stps", bufs=2, space="PSUM"))
        accp = ctx.enter_context(tc.tile_pool(name="accps", bufs=1, space="PSUM"))

        # X^T: xt[p, c, t, nb] = X^T[t*128+p, c*1024+nb] (DMA-chunk major)
        xt = big.tile([P, XC * DT * XCR], bf16)
        xt4 = xt[:].rearrange("p (c t nb) -> p c t nb", c=XC, t=DT)

        # chunk-0's first two d-tiles are the very first trigger (the
        # Scalar queue fires it ~4us before Sync finishes its preamble;
        # bulk rate is DGE-bound either way but every early block helps).
        nc.scalar.dma_start(xt4[:, 0, 0:2], XT[0][:, 0:2])

        ones_sq = cpool.tile([P, P], bf16)
        nc.vector.memset(ones_sq[:], 1.0)

        # ---- HAM warmup: dummy matmuls into the out-accumulator PSUM bank.
        # They only depend on the memset, so they run during the ~9us DMA
        # startup latency; the first real O matmul has start=True and
        # overwrites. Keeps the PE clock-gate at 2.4GHz for the real work.
        warm_rhs = cpool.tile([P, FB], bf16)
        nc.vector.memset(warm_rhs[:], 1.0)
        warm_ps = accp.tile([P, qc], f32, tag="out", name="warm")
        for _ in range(WARM_MM):
            nc.tensor.matmul(warm_ps[:, 0:FB], ones_sq[:], warm_rhs[:],
                             start=True, stop=True)

        w_sb = {}

        def load_w(name, eng=None):
            t = cpool.tile([P, DT * H], bf16, tag=name, name=f"w_{name}")
            (eng or nc.sync).dma_start(
                t[:].rearrange("p (t h) -> p t h", t=DT), Ws[name])
            w_sb[name] = t

        # fp8 K-projection operands (DoubleRow, 256-deep contraction):
        # wk8[p, u, i, h] = Wk[(2u+i)*128+p, h]; per-chunk X^T likewise.
        wk8 = cpool.tile([P, UP * 2 * H], f8, name="wk8")
        wk8_4 = wk8[:].rearrange("p (u i h) -> p u i h", u=UP, i=2)

        def load_wk8():
            # flat AP: one contiguous 1KB line per partition (the 4D view
            # would fragment the DMA into 128B lines)
            nc.sync.dma_start(wk8[:], WK8)
        xf8p = ctx.enter_context(tc.tile_pool(name="xf8", bufs=3))
        x8_sb = {}

        def produce_f8(c):
            t = xf8p.tile([P, UP * 2 * XCR], f8, tag="x8", name=f"x8_{c}")
            # flat DMA (4KB contiguous per-partition lines); the matmul
            # uses the 4D view of the same tile. Triggered from the
            # GpSimd queue: Sync's FIFO blocks on semaphore-reuse waits
            # for earlier chunk DMAs, which would delay these by ~8us.
            nc.gpsimd.dma_start(t[:], XF8[c])
            x8_sb[c] = t[:].rearrange("p (u i nb) -> p u i nb", u=UP, i=2)

        def xt_job(hc, t):
            """[128, 512] X^T slice for projection job hc, d-tile t."""
            c = hc * CR // XCR
            o = (hc * CR) % XCR
            return xt4[:, c, t, o:o + CR]
        kT = big.tile([P, n], bf16)          # K^T[h, keys]
        qT = big.tile([P, nq], bf16)         # Q^T[h, q]
        v_sb = big.tile([P, NT * H], bf16)   # V[k%128, kt*H + h]
        v_sb3 = v_sb[:].rearrange("p (kt h) -> p kt h", h=H)
        # exp ring: pT3[:, r, :] = P^T slice for k-tile with kt % PR == r
        pT_all = big.tile([P, PR * qc], bf16)
        pT3 = pT_all[:].rearrange("p (r q) -> p r q", r=PR)

        def produce_data(c):
            nc.sync.dma_start(xt4[:, c], XT[c])

        def produce_slice(c, h2):
            """4-d-tile (512KB) half of chunk c: fine-grained arrival
            while keeping 4KB per-partition DMA lines (2KB lines halve
            the early DMA bandwidth)."""
            nc.sync.dma_start(xt4[:, c, 4 * h2:4 * h2 + 4],
                              XT[c][:, 4 * h2:4 * h2 + 4])

        def proj_pair_stages(jobs, on_scalar=False):
            """Return 4 closures, each emitting 2 t-steps of the pair's
            interleaved matmuls; the last also emits copies/transposes."""
            state = {}

            def stage(si):
                def run():
                    if si == 0:
                        state['tiles'] = [
                            p12.tile([P, CR], f32, tag="pps",
                                     name=f"ps_{w}{c}")
                            for w, c in jobs]
                    for (wname, c), ps in zip(jobs, state['tiles']):
                        if F8K and wname == "wk" and c >= 1:
                            # fp8 DoubleRow: one 256-deep matmul per stage
                            # (keys 512+ take the fast path; K0/Q/V stay
                            # bf16 to bound the accuracy cost)
                            nc.tensor.matmul(
                                ps[:], wk8_4[:, si], x8_sb[c][:, si],
                                start=(si == 0), stop=(si == 3),
                                perf_mode=DR,
                            )
                        else:
                            for t in range(si * 2, si * 2 + 2):
                                nc.tensor.matmul(
                                    ps[:],
                                    w_sb[wname][:, t * H:(t + 1) * H],
                                    xt_job(c, t),
                                    start=(t == 0),
                                    stop=(t == DT - 1),
                                )
                    if si == 3:
                        for ji, ((wname, c), ps) in enumerate(
                                zip(jobs, state['tiles'])):
                            # split the pair's two PSUM-evacuation copies
                            # across ACT (job 0) and DVE (job 1) so neither
                            # engine eats both and the p12 slots free fast
                            cp = (nc.scalar.copy if ji == 0
                                  else nc.vector.tensor_copy)
                            if wname == "wv":
                                vt = vtp.tile([P, CR], bf16, tag="vt",
                                              name=f"vt{c}")
                                cp(vt[:], ps[:])
                                nc.sync.dma_start_transpose(
                                    v_sb3[:, c * KPC:(c + 1) * KPC], vt[:])
                            else:
                                dst = kT if wname == "wk" else qT
                                cp(dst[:, c * CR:(c + 1) * CR], ps[:])
                return run
            return [stage(i) for i in range(4)]

        def proj_pair(jobs, on_scalar=False):
            for s in proj_pair_stages(jobs, on_scalar):
                s()

        # ---- Phase 1: weights first, then X chunks fine-grained/early.
        # All DMA triggers are issued up front in need-order; the DGE
        # queues execute them FIFO so later chunks never delay earlier
        # ones, and the first projection matmul only waits for
        # wk + wv + one 2-t-slice of chunk 0 (~0.75MB).
        # chunk-0's first piece was already triggered from the Scalar
        # queue above; wk rides Scalar too, the rest go on Sync in
        # consumption order (2-d-tile pieces for chunk 0 so the solo K0
        # chain starts on ~0.5MB of data).
        load_w("wk", eng=nc.scalar)
        load_w("wv")
        for j2 in (1, 2, 3):
            nc.sync.dma_start(xt4[:, 0, 2 * j2:2 * j2 + 2],
                              XT[0][:, 2 * j2:2 * j2 + 2])
        load_w("wq")
        produce_slice(1, 0)
        produce_slice(1, 1)
        if F8K:
            load_wk8()
            # Gate the gpsimd-issued fp8 chunk-1 DMA behind the wq load:
            # an ungated gpsimd trigger fires at ~6us and its 0.5MB steals
            # DMA bandwidth from the critical prefix chain (c0/weights/c1).
            gate = cpool.tile([P, 2], bf16, name="gate")
            nc.gpsimd.tensor_copy(gate[:, 0:1], w_sb["wq"][:, 0:1])
            produce_f8(1)
        for c in range(2, XC):
            produce_data(c)
        def proj_single(wname, c):
            """Single-job 8-matmul chain for phase 1: each job starts as
            soon as ITS data has landed (a K/V pair's first stage would
            wait for wk+wv+half of chunk 0 at once). Copies on ACT (idle
            pre-attention)."""
            ps = p12.tile([P, CR], f32, tag="pps", name=f"ps1_{wname}{c}")
            for t in range(DT):
                nc.tensor.matmul(
                    ps[:], w_sb[wname][:, t * H:(t + 1) * H], xt_job(c, t),
                    start=(t == 0), stop=(t == DT - 1))
            if wname == "wv":
                vt = vtp.tile([P, CR], bf16, tag="vt", name=f"vt{c}")
                nc.scalar.copy(vt[:], ps[:])
                nc.sync.dma_start_transpose(
                    v_sb3[:, c * KPC:(c + 1) * KPC], vt[:])
            else:
                dst = kT if wname == "wk" else qT
                nc.scalar.copy(dst[:, c * CR:(c + 1) * CR], ps[:])

        proj_single("wk", 0)
        proj_single("wv", 0)
        proj_single("wq", 0)
        proj_single("wq", 1)

        def emit_S(q0, kt):
            st = stp.tile([P, qc], f32, tag="st", name=f"st{q0}_{kt}")
            for j in range(0, qc, FB):
                nc.tensor.matmul(
                    st[:, j:j + FB],
                    kT[:, kt * P:(kt + 1) * P],
                    qT[:, q0 + j:q0 + j + FB],
                    start=True, stop=True,
                )
            return st

        # deferred epilogue state from the previous q-chunk
        pending = {}

        def defer_tree_a():
            # acc2 = acc4[0]+acc4[1] | acc4[2]+acc4[3]  (interleaved pairs)
            if not pending:
                return
            a4 = pending['acc4']
            acc2 = epp.tile([P, 2 * qc], bf16, tag="acc2", bufs=1,
                            name="acc2")
            nc.vector.tensor_add(acc2[:], a4[:, 0:2 * qc], a4[:, 2 * qc:])
            pending['acc2'] = acc2

        def defer_tree_b():
            if not pending:
                return
            acc2 = pending.pop('acc2')
            accf = epp.tile([P, qc], bf16, tag="accf", bufs=1, name="accf")
            nc.vector.tensor_add(accf[:], acc2[:, 0:qc], acc2[:, qc:])
            pending['accf'] = accf

        def finish_epilogue():
            if not pending:
                return
            accf, ob, q0p = pending.pop('accf'), pending.pop('ob'), \
                pending.pop('q0')
            pending.pop('acc4')
            l_a = p12.tile([P, FB], f32, tag="pps", name=f"la{q0p}")
            l_b = p12.tile([P, FB], f32, tag="pps", name=f"lb{q0p}")
            r_sb = epp.tile([P, qc], f32, tag="rsb", name=f"rsb{q0p}")
            o_sb = epp.tile([P, qc], f32, tag="osb", name=f"osb{q0p}")
            nc.tensor.matmul(l_a[:], ones_sq[:], accf[:, 0:FB],
                             start=True, stop=True)
            nc.vector.reciprocal_approx_fast(r_sb[:, 0:FB], l_a[:])
            nc.tensor.matmul(l_b[:], ones_sq[:], accf[:, FB:qc],
                             start=True, stop=True)
            nc.gpsimd.tensor_mul(o_sb[:, 0:FB], ob[:, 0:FB], r_sb[:, 0:FB])
            nc.sync.dma_start(OT[:, q0p:q0p + FB], o_sb[:, 0:FB])
            nc.vector.reciprocal_approx_fast(r_sb[:, FB:qc], l_b[:])
            nc.vector.tensor_mul(o_sb[:, FB:qc], ob[:, FB:qc], r_sb[:, FB:qc])
            nc.sync.dma_start(OT[:, q0p + FB:q0p + qc], o_sb[:, FB:qc])

        for qi in range(NQC):
            q0 = qi * qc
            final = (qi == NQC - 1)
            actions = {}
            if qi == 0:
                if F8K:
                    for kt_i, c in zip((0, 2, 6, 10, 14, 18), range(2, NC)):
                        actions.setdefault(kt_i, []).append(
                            (produce_f8, (c,)))
                pjobs = [(("wk", c), ("wv", c)) for c in range(1, NC)]
                pjobs.append((("wq", 2), ("wq", 3)))
                # K1/V1 compressed into the first two slots (needed by S(4))
                # All in-loop proj copies run on ACT (exp leaves ~700ns/kt
                # of ACT slack in qi0) so the DVE never gates the p12
                # PSUM rotation from behind a 2.3us denominator add.
                s10, s11, s12, s13 = proj_pair_stages(pjobs[0],
                                                      on_scalar=True)
                actions.setdefault(0, []).extend([(s10, ()), (s11, ())])
                actions.setdefault(1, []).extend([(s12, ()), (s13, ())])
                at = 2
                for jobs in pjobs[1:]:
                    for s in proj_pair_stages(jobs, on_scalar=True):
                        actions.setdefault(at, []).append((s, ()))
                        at += 1
            else:
                actions.setdefault(1, []).append((defer_tree_a, ()))
                actions.setdefault(2, []).append((defer_tree_b, ()))
                actions.setdefault(5, []).append((finish_epilogue, ()))

            out_ps = accp.tile([P, qc], f32, tag="out", name=f"out{qi}")
            st_tiles = {0: emit_S(q0, 0)}
            # denominator accumulator: [p, 4, qc] bf16, four interleaved
            # partial sums tree-reduced on DVE before the epilogue
            # ones-matmul (partition reduction only).
            acc4 = gsp.tile([P, GL * qc], bf16, tag="a4", name=f"a4_{qi}")
            fin = {}  # final-chunk epilogue tiles
            for kt in range(NT):
                # S(kt+1) is emitted FIRST so it can never queue behind a
                # stalled projection matmul in the PE FIFO (the proj
                # pipeline is gated on copies; S leading breaks the
                # proj->copy->exp->S convoy cycle).
                if kt + 1 < NT:
                    st_tiles[kt + 1] = emit_S(q0, kt + 1)
                # exp on ACT into the ring
                nc.scalar.activation(
                    pT3[:, kt % PR, :], st_tiles.pop(kt)[:],
                    AF.Exp, scale=SCALE)
                # O^T accumulation for the PREVIOUS kt (software pipeline)
                if kt > 0:
                    for j in range(0, qc, FB):
                        nc.tensor.matmul(
                            out_ps[:, j:j + FB],
                            v_sb3[:, kt - 1, :],
                            pT3[:, (kt - 1) % PR, j:j + FB],
                            start=(kt - 1 == 0), stop=False,
                        )
                # actions (proj stages incl. their DVE V-copies) BEFORE the
                # denominator adds: a V-copy queued behind a group-add that
                # waits on future exps closes a cross-engine convoy cycle
                # (O-ldweights <- v-transpose <- v-copy <- group-add <- exp
                # <- S behind the blocked O in the PE FIFO).
                for fn, arg in actions.get(kt, ()):
                    fn(*arg)
                # denominator: one contiguous 4-slice DVE add per 4 k-tiles.
                # The final q-chunk keeps its last 8 k-tiles out of the
                # accumulator and spreads its tree reduction across kts
                # 25-31 (each step gated only on an already-finished exp)
                # so no multi-us DVE convoy forms at the very end.
                ngrp = NT - 2 * GL if final else NT
                if kt < ngrp and kt % GL == GL - 1:
                    r0 = (kt - (GL - 1)) % PR
                    grp = pT_all[:, r0 * qc:(r0 + GL) * qc]
                    if kt == GL - 1:
                        nc.vector.tensor_copy(acc4[:], grp)
                    else:
                        nc.vector.tensor_add(acc4[:], acc4[:], grp)
                if final:
                    if kt == 25:
                        a2 = epp.tile([P, 2 * qc], bf16, tag="acc2f",
                                      bufs=1, name="acc2f")
                        nc.vector.tensor_add(
                            a2[:], acc4[:, 0:2 * qc], acc4[:, 2 * qc:])
                        fin['a2'] = a2
                    elif kt == 26:
                        af = epp.tile([P, qc], bf16, tag="accff", bufs=1,
                                      name="accff")
                        nc.vector.tensor_add(
                            af[:], fin['a2'][:, 0:qc], fin['a2'][:, qc:])
                        fin['af'] = af
                    elif kt in (27, 28, 29):
                        # pair-sums of raw ring slices (24,25),(26,27),
                        # (28,29) — each emitted the iteration after its
                        # second exp, then folded into a running total so
                        # the final PE chain is one matmul per half
                        k0 = 24 + 2 * (kt - 27)
                        pt = epp.tile([P, qc], bf16, tag=f"pp{k0}",
                                      bufs=1, name=f"pp{k0}")
                        nc.vector.tensor_add(
                            pt[:], pT3[:, k0 % PR, :],
                            pT3[:, (k0 + 1) % PR, :])
                        fin[f'p{k0}'] = pt
                        if kt == 28:
                            tt = epp.tile([P, qc], bf16, tag="tt",
                                          bufs=1, name="tt")
                            nc.vector.tensor_add(
                                tt[:], fin['af'][:], fin['p24'][:])
                            fin['tt'] = tt
                        elif kt == 29:
                            nc.vector.tensor_add(
                                fin['tt'][:], fin['tt'][:], fin['p26'][:])
                    elif kt == 30:
                        # tt now holds slices 0-29; pT(30) stays a PE
                        # pre-chain term (it overlaps the exp stream there;
                        # a DVE fold would serialize exp(30)->add->l-MMs)
                        nc.vector.tensor_add(
                            fin['tt'][:], fin['tt'][:], fin['p28'][:])

            if not final:
                # last O^T tile, evacuate on DVE (ACT stays exp-only),
                # defer the l/recip/scale chain into the next q-chunk.
                for j in range(0, qc, FB):
                    nc.tensor.matmul(
                        out_ps[:, j:j + FB],
                        v_sb3[:, NT - 1, :],
                        pT3[:, (NT - 1) % PR, j:j + FB],
                        start=False, stop=True,
                    )
                ob = epp.tile([P, qc], f32, tag="ob", name=f"ob{qi}")
                nc.vector.tensor_copy(ob[:], out_ps[:])
                pending.update(acc4=acc4, ob=ob, q0=q0)
            else:
                # ---- final-chunk tail: everything that can run before
                # exp(31) is emitted first; the post-exp(31) chain is
                # [O31-half, l-last-matmul] x2 -> recip (on ACT, free by
                # then) -> DVE mul -> DMA, quartered so scale and
                # DMA-out overlap.
                l_a = p12.tile([P, FB], f32, tag="pps", name="la_f")
                l_b = p12.tile([P, FB], f32, tag="pps", name="lb_f")
                tt = fin['tt']
                p30 = pT3[:, (NT - 2) % PR, :]
                for g, t in enumerate((tt, p30)):
                    nc.tensor.matmul(l_a[:], ones_sq[:], t[:, 0:FB],
                                     start=(g == 0), stop=False)
                for g, t in enumerate((tt, p30)):
                    nc.tensor.matmul(l_b[:], ones_sq[:], t[:, FB:qc],
                                     start=(g == 0), stop=False)
                last = pT3[:, (NT - 1) % PR, :]
                r_sb = epp.tile([P, qc], f32, tag="rsb", name="rsb_f")
                o_sb = epp.tile([P, qc], f32, tag="osb", name="osb_f")
                # Post-exp(31) PE order: l-lasts FIRST (they gate the
                # recips, the longest downstream chain), then the O31
                # halves; all out_ps writes still precede any out_ps read
                # (Tile serializes PSUM write-after-read at tile
                # granularity).
                nc.tensor.matmul(l_a[:], ones_sq[:], last[:, 0:FB],
                                 start=False, stop=True)
                nc.tensor.matmul(l_b[:], ones_sq[:], last[:, FB:qc],
                                 start=False, stop=True)
                for j in (0, FB):
                    nc.tensor.matmul(
                        out_ps[:, j:j + FB],
                        v_sb3[:, NT - 1, :],
                        pT3[:, (NT - 1) % PR, j:j + FB],
                        start=False, stop=True,
                    )
                # Half A is evacuated by ACT (free after the last exp) and
                # scaled on GpSimd while DVE recips + scales half B from
                # PSUM; DMA triggers ride the idle Scalar/GpSimd queues so
                # they don't serialize behind each other on Sync.
                ob_f = epp.tile([P, FB], f32, tag="ob", name="ob_f")
                nc.scalar.copy(ob_f[:], out_ps[:, 0:FB])
                nc.vector.reciprocal_approx_fast(r_sb[:, 0:FB], l_a[:])
                nc.vector.reciprocal_approx_fast(r_sb[:, FB:qc], l_b[:])
                h = FB // 2
                # gpsimd's dma triggers cost ~670ns and queue behind its
                # muls — put its halves' OT triggers on Sync (idle here)
                nc.gpsimd.tensor_mul(o_sb[:, 0:h], ob_f[:, 0:h],
                                     r_sb[:, 0:h])
                nc.sync.dma_start(OT[:, q0:q0 + h], o_sb[:, 0:h])
                nc.vector.tensor_mul(o_sb[:, FB:FB + h],
                                     out_ps[:, FB:FB + h],
                                     r_sb[:, FB:FB + h])
                nc.scalar.dma_start(OT[:, q0 + FB:q0 + FB + h],
                                    o_sb[:, FB:FB + h])
                nc.gpsimd.tensor_mul(o_sb[:, h:FB], ob_f[:, h:FB],
                                     r_sb[:, h:FB])
                nc.sync.dma_start(OT[:, q0 + h:q0 + FB], o_sb[:, h:FB])
                nc.vector.tensor_mul(o_sb[:, FB + h:qc],
                                     out_ps[:, FB + h:qc],
                                     r_sb[:, FB + h:qc])
                nc.scalar.dma_start(OT[:, q0 + FB + h:q0 + qc],
                                    o_sb[:, FB + h:qc])

        finish_epilogue()


def build_bass(n=N, d=D, nq=NQ):
    import concourse.mybir as mybir
    from concourse import bacc
    from concourse.tile import TileContext

    dt = mybir.dt
    nc = bacc.Bacc("TRN2", target_bir_lowering=False, debug=False)
    XT = nc.dram_tensor(
        "XT", [XC, P, DT, XCR], dt.bfloat16, kind="ExternalInput").ap()
    XF8 = nc.dram_tensor(
        "XF8", [XC, P, UP * 2 * XCR], dt.float8e4, kind="ExternalInput").ap()
    Ws = {}
    for name in ("wq", "wk", "wv"):
        Ws[name] = nc.dram_tensor(
            name.upper(), [P, DT, H], dt.bfloat16, kind="ExternalInput").ap()
    WK8 = nc.dram_tensor(
        "WK8", [P, UP * 2 * H], dt.float8e4, kind="ExternalInput").ap()
    OT = nc.dram_tensor("OT", [H, nq], dt.float32, kind="ExternalOutput").ap()

    with TileContext(nc) as tc:
        emit_attention(tc, XT, Ws, OT, XF8=XF8, WK8=WK8, n=n, d=d, nq=nq)
    nc.compile()  # bacc passes: split multi-waits into EVSEM chains, etc.
    return nc


_CACHED = {}


def _get_nc():
    if "nc" not in _CACHED:
        _CACHED["nc"] = build_bass()
    return _CACHED["nc"]


def _prep_w(w):
    import ml_dtypes
    # [D, H] f32 -> [128, DT, H] bf16 with w_t[p, t, h] = W[t*128+p, h]
    return np.ascontiguousarray(
        w.reshape(DT, P, H).transpose(1, 0, 2)).astype(ml_dtypes.bfloat16)


def _prep_xt(xb):
    import ml_dtypes
    # [N, D] f32 -> [XC, 128, DT, XCR] bf16:
    # XT[c, p, t, nb] = X[c*XCR+nb, t*128+p]
    x4 = xb.reshape(XC, XCR, DT, P)          # [c, nb, t, p]
    return np.ascontiguousarray(
        x4.transpose(0, 3, 2, 1)).astype(ml_dtypes.bfloat16)


def _prep_xf8(xb):
    import ml_dtypes
    # [N, D] f32 -> [XC, 128, UP, 2, XCR] e4m3 with
    # XF8[c, p, u, i, nb] = X[c*XCR+nb, (2u+i)*128+p]
    x5 = xb.reshape(XC, XCR, UP, 2, P)       # [c, nb, u, i, p]
    return np.ascontiguousarray(
        x5.transpose(0, 4, 2, 3, 1)).astype(
            ml_dtypes.float8_e4m3fn).reshape(XC, P, UP * 2 * XCR)


def _prep_wk8(w):
    import ml_dtypes
    # [D, H] f32 -> [128, UP, 2, H] e4m3 with
    # WK8[p, u, i, h] = W[(2u+i)*128+p, h]
    w4 = w.reshape(UP, 2, P, H)              # [u, i, p, h]
    return np.ascontiguousarray(
        w4.transpose(2, 0, 1, 3)).astype(
            ml_dtypes.float8_e4m3fn).reshape(P, UP * 2 * H)


def kernel(X, Wq, Wk, Wv, trace=False):
    """Full-input entry point: X [4,4096,1024] f32 -> [4,4096,128] f32."""
    from concourse.bass_utils import run_bass_kernel_spmd

    X = np.ascontiguousarray(X, dtype=np.float32)
    wmap = {"WQ": _prep_w(np.asarray(Wq, dtype=np.float32)),
            "WK": _prep_w(np.asarray(Wk, dtype=np.float32)),
            "WV": _prep_w(np.asarray(Wv, dtype=np.float32)),
            "WK8": _prep_wk8(np.asarray(Wk, dtype=np.float32))}

    nc = _get_nc()
    in_maps = []
    for core in range(NCORES):
        b, half = core // QSPLIT, core % QSPLIT
        xb = X[b]
        if half:
            # roll so this core's queries are rows [0:NQ); key set is unchanged
            xb = np.concatenate([xb[NQ:], xb[:NQ]], axis=0)
        in_maps.append({"XT": _prep_xt(xb), "XF8": _prep_xf8(xb), **wmap})

    res = run_bass_kernel_spmd(
        nc, in_maps, core_ids=list(range(NCORES)), trace=trace
    )
    out = np.empty((B, N, H), dtype=np.float32)
    for core in range(NCORES):
        b, half = core // QSPLIT, core % QSPLIT
        out[b, half * NQ:(half + 1) * NQ] = res.results[core]["OT"].T
    if trace:
        return out, res
    return out
